# revision 13
# baseline (speedup 1.0000x reference)
"""BiLSTM-CRF loss kernel for 8 Trainium2 NeuronCores (v2).

Sharding: data-parallel over batch (64 -> 8 cores x 8 rows). Each core runs
both LSTM directions for its batch shard, computes CRF emissions, runs the
CRF forward pass in the exp domain, the gold-path score, and writes the
partial sum of (forward - gold) over its 8 rows. Host sums partials / 64.

v2 changes vs v1:
  - Single gather/transpose set: backward direction consumes forward-time
    xpT at reversed indices and writes hsT at reversed slots, so hs_b ends
    up in forward-time order for the emission matmuls.
  - Input projection xp is injected into the LSTM gate PSUM via an
    identity-stationary matmul, eliminating the per-step DVE add; tanh
    reads PSUM directly on the Activation engine.
  - The v gate product runs on GpSimd, balancing DVE.
  - P1 bias-adds and P3's exp(feats) run on the Activation engine with
    per-partition bias (bout folded into the exp).
  - No max-shift in the CRF: E = exp(W h + b) directly; periodic renorm
    keeps the exp-domain scan in f32 range.
  - CRF scan in bf16 (single HW matmul per step instead of an fp32 pair),
    split into two staggered 4-row chains with renorm every 16 steps.
"""

import sys

sys.path.insert(0, "/opt/trn_rl_repo")

import numpy as np
import ml_dtypes

import concourse.bass as bass
from concourse import bacc
import concourse.tile as tile
from concourse import mybir
from concourse import bass_isa
from concourse.bass import IndirectOffsetOnAxis
from concourse.bass_utils import run_bass_kernel_spmd
from concourse.masks import make_identity

F32 = mybir.dt.float32
BF16 = mybir.dt.bfloat16
I32 = mybir.dt.int32
ALU = mybir.AluOpType
AF = mybir.ActivationFunctionType
AX = mybir.AxisListType

B, L, E, H, C = 64, 256, 256, 256, 20
G = 4 * H
NCORES = 8
BC = B // NCORES            # batch rows per core
CH = 8                      # gate-hidden chunks of 128 (c = gate*2 + half)
NT = (L * BC) // 128        # token tiles = 16
TPT = 128 // BC             # timesteps per token tile = 16
REN = 16                    # CRF renorm interval (per chain)
NREN = L // REN             # 16
HBC = BC // 2               # rows per CRF chain = 4
START, STOP = 18, 19

_CACHE = {}
DEBUG = False
SPEC_W = 6     # LSTM speculative warmup steps
CSPEC_W = 6    # CRF speculative warmup steps
NH = L // 2     # 128


def _build_module():
    nc = bacc.Bacc(None, target_bir_lowering=False, debug=False)

    # ---- DRAM I/O ----
    d_embed = nc.dram_tensor("embed_bf", [50000, E], BF16, kind="ExternalInput")
    d_idxf = nc.dram_tensor("idx_f", [128, NT], I32, kind="ExternalInput")
    d_wih = nc.dram_tensor("wih", [128, 2, 2, CH, 128], BF16, kind="ExternalInput")
    d_whh = nc.dram_tensor("whh", [128, 2, 2, CH, 128], BF16, kind="ExternalInput")
    d_xbias = nc.dram_tensor("xbias", [128, 2, CH], F32, kind="ExternalInput")
    d_h0 = nc.dram_tensor("h0T", [128, 2, 2, BC], BF16, kind="ExternalInput")
    d_c0 = nc.dram_tensor("c0T", [128, 2, 2, BC], F32, kind="ExternalInput")
    d_wout = nc.dram_tensor("woutT", [128, 2, 2, C], BF16, kind="ExternalInput")
    d_bout = nc.dram_tensor("bout", [C, 1], F32, kind="ExternalInput")
    d_transT = nc.dram_tensor("transT", [C, C], F32, kind="ExternalInput")
    d_transTb = nc.dram_tensor("transTb", [C, C], BF16, kind="ExternalInput")
    d_tstop = nc.dram_tensor("tstop", [C, 1], F32, kind="ExternalInput")
    d_ohprev = nc.dram_tensor("ohprev", [C, BC, L], BF16, kind="ExternalInput")
    d_ohcur = nc.dram_tensor("ohcur", [C, BC, L], F32, kind="ExternalInput")
    d_ohcur_tb = nc.dram_tensor("ohcur_tb", [C, L, BC], F32, kind="ExternalInput")
    d_ohlast = nc.dram_tensor("ohlast", [C, BC], F32, kind="ExternalInput")
    d_a0 = nc.dram_tensor("a0", [C, BC], BF16, kind="ExternalInput")
    d_gold = nc.dram_tensor("gold_out", [1, BC], F32, kind="ExternalOutput")
    d_paf = nc.dram_tensor("paf_out", [1, BC], F32, kind="ExternalOutput")
    d_sall = nc.dram_tensor("sall_out", [1, BC, NREN], F32, kind="ExternalOutput")
    if DEBUG:
        d_dbg_eT = nc.dram_tensor("dbg_eT", [C, L, BC], F32, kind="ExternalOutput")
        d_dbg_gsum = nc.dram_tensor("dbg_gsum", [C, BC], F32, kind="ExternalOutput")
        d_dbg_gold = nc.dram_tensor("dbg_gold", [1, BC], F32, kind="ExternalOutput")
        d_dbg_hf = nc.dram_tensor("dbg_hf", [128, 2, BC], BF16, kind="ExternalOutput")
        d_dbg_hb = nc.dram_tensor("dbg_hb", [128, 2, BC], BF16, kind="ExternalOutput")
        d_dbg_sall = nc.dram_tensor("dbg_sall", [1, BC, NREN], F32, kind="ExternalOutput")
        d_dbg_xp = nc.dram_tensor("dbg_xp", [128, 8, CH, BC], BF16, kind="ExternalOutput")
        d_dbg_cst = nc.dram_tensor("dbg_cst", [128, 2, 2, BC], F32, kind="ExternalOutput")

    with tile.TileContext(nc) as tc:
        with (
            tc.tile_pool(name="persist", bufs=1) as pp,
            tc.tile_pool(name="work", bufs=3) as wp,
            tc.tile_pool(name="lstm", bufs=3) as lp,
        ):
            # ---- persistent SBUF ----
            wih_sb = pp.tile([128, 2, 2, CH, 128], BF16, tag="wih")
            whh_sb = pp.tile([128, 2, 2, CH, 128], BF16, tag="whh")
            xbias_sb = pp.tile([128, 2, CH], F32, tag="xbias")
            wout_sb = pp.tile([128, 2, 2, C], BF16, tag="wout")
            bout_sb = pp.tile([C, 1], F32, tag="bout")
            transT_sb = pp.tile([C, C], F32, tag="transT")
            transTb_sb = pp.tile([C, C], BF16, tag="transTb")
            tstop_sb = pp.tile([C, 1], F32, tag="tstop")
            ohprev_sb = pp.tile([C, BC, L], BF16, tag="ohprev")
            ohcur_sb = pp.tile([C, BC, L], F32, tag="ohcur")
            ohcur_tb_sb = pp.tile([C, L, BC], F32, tag="ohcur_tb")
            ohlast_sb = pp.tile([C, BC], F32, tag="ohlast")
            idxf_sb = pp.tile([128, NT], I32, tag="idxf")
            ident128 = pp.tile([128, 128], BF16, tag="id128")
            xTf = pp.tile([128, 2, NT, 128], BF16, tag="xTf")
            # xp^T: [ghid-part, t, chunk, b]  (both dirs in forward time order)
            xpT = [pp.tile([128, L, CH, BC], BF16, name=f"xpT{d}", tag=f"xpT{d}") for d in (0, 1)]
            # H history: [hid-part, k-half, slot(0..L), b]
            # dir0: init slot 0, step s reads s, writes s+1 (h_f[t] at slot t+1)
            # dir1: init slot L, step s reads L-s, writes L-1-s (h_b[t] at slot t)
            hsT = [pp.tile([128, 2, L + 1, BC], BF16, name=f"hsT{d}", tag=f"hsT{d}") for d in (0, 1)]
            cst = [[pp.tile([128, 2, BC], F32, name=f"cst{d}{hf}", tag=f"cst{d}{hf}")
                    for hf in (0, 1)] for d in (0, 1)]
            hwu = [pp.tile([128, 2, 2, BC], BF16, name=f"hwu{d}", tag=f"hwu{d}") for d in (0, 1)]
            zro = pp.tile([128, 2, BC], BF16, tag="zro")
            eT = pp.tile([C, L, BC], F32, tag="eT")
            pplus = pp.tile([C, C], BF16, tag="pplus")
            wstop = pp.tile([C, 1], BF16, tag="wstop")
            # applied renorm scales (exact f32 bookkeeping): [1, chain*HBC, NREN]
            sall = pp.tile([1, BC, NREN], F32, tag="sall")
            ones1 = pp.tile([1, C], F32, tag="ones1")
            ones20c = pp.tile([C, 1], BF16, tag="ones20c")
            ones20f = pp.tile([C, 1], F32, tag="ones20f")
            avec = [pp.tile([C, 2, BC], BF16, name=f"avec{x}", tag=f"avec{x}") for x in range(4)]
            gsum = pp.tile([C, BC], F32, tag="gsum")
            gold_sb = pp.tile([1, BC], F32, tag="gold")
            cnt_sb = pp.tile([C, BC], F32, tag="cnt")

            # ---- load constants ----
            nc.sync.dma_start(out=wih_sb[:], in_=d_wih[:])
            nc.sync.dma_start(out=whh_sb[:], in_=d_whh[:])
            nc.sync.dma_start(out=xbias_sb[:], in_=d_xbias[:])
            nc.sync.dma_start(out=wout_sb[:], in_=d_wout[:])
            nc.sync.dma_start(out=bout_sb[:], in_=d_bout[:])
            nc.sync.dma_start(out=transT_sb[:], in_=d_transT[:])
            nc.sync.dma_start(out=transTb_sb[:], in_=d_transTb[:])
            nc.sync.dma_start(out=tstop_sb[:], in_=d_tstop[:])
            nc.sync.dma_start(out=ohprev_sb[:], in_=d_ohprev[:])
            nc.sync.dma_start(out=ohcur_sb[:], in_=d_ohcur[:])
            nc.sync.dma_start(out=ohcur_tb_sb[:], in_=d_ohcur_tb[:])
            nc.sync.dma_start(out=ohlast_sb[:], in_=d_ohlast[:])
            nc.sync.dma_start(out=idxf_sb[:], in_=d_idxf[:])
            nc.sync.dma_start(out=hsT[0][:, :, 0, :], in_=d_h0[:, 0, :, :])
            nc.sync.dma_start(out=hsT[1][:, :, L, :], in_=d_h0[:, 1, :, :])
            for d in (0, 1):
                nc.sync.dma_start(out=cst[d][0][:], in_=d_c0[:, d, :, :])
                nc.vector.memset(cst[d][1][:], 0.0)
            nc.vector.memset(zro[:], 0.0)
            make_identity(nc, ident128[:])
            nc.vector.memset(ones1[:], 1.0)
            nc.vector.memset(ones20c[:], 1.0)
            nc.vector.memset(ones20f[:], 1.0)

            # P+ = exp(transT) in bf16;  wstop = exp(T[STOP,:]) in bf16
            nc.scalar.activation(pplus[:], transT_sb[:], AF.Exp)
            nc.scalar.activation(wstop[:], tstop_sb[:], AF.Exp)

            # A0: time-chain 0 starts at onehot(START); chains 1-3 warm up
            # from uniform over CSPEC_W steps, junction-renormalized.
            nc.vector.memset(avec[0][:], 0.0)
            nc.sync.dma_start(out=avec[0][:, 0, :], in_=d_a0[:])
            for x in (1, 2, 3):
                nc.vector.memset(avec[x][:], 1.0)

            # ---- gold transition score (inputs only; runs during P1) ----
            # pu = trans @ ohprev ; gsum = sum_t (pu * ohcur)
            ps_gold = tc.tile_pool(name="ps_gold", bufs=1, space="PSUM")
            psG = ps_gold.__enter__()
            pu = psG.tile([C, BC * L], F32, tag="pu")
            for n in range(4):
                nc.tensor.matmul(
                    pu[:, n * 512 : (n + 1) * 512],
                    transTb_sb[:],
                    ohprev_sb[:].rearrange("p b t -> p (b t)")[
                        :, n * 512 : (n + 1) * 512
                    ],
                    start=True,
                    stop=True,
                )
            prod = pp.tile([C, BC, L], F32, tag="prod")
            nc.vector.scalar_tensor_tensor(
                out=prod[:].rearrange("p b t -> p (b t)"), in0=pu[:], scalar=0.0,
                in1=ohcur_sb[:].rearrange("p b t -> p (b t)"), op0=ALU.add, op1=ALU.mult,
            )
            nc.vector.tensor_reduce(out=gsum[:], in_=prod[:], axis=AX.X, op=ALU.add)
            # cnt[c,b] = #timesteps with tag c (for the bout term of emissions)
            nc.vector.tensor_reduce(out=cnt_sb[:], in_=ohcur_sb[:], axis=AX.X, op=ALU.add)
            # cnt *= bout (per-tag emission bias counts); gsum += cnt
            nc.gpsimd.tensor_tensor(
                out=cnt_sb[:], in0=cnt_sb[:],
                in1=bout_sb[:].to_broadcast([C, BC]), op=ALU.mult,
            )
            nc.gpsimd.tensor_add(gsum[:], gsum[:], cnt_sb[:])
            ps_gold.__exit__(None, None, None)

            # ---- P1: gather + transpose + input projection ----
            # Projection groups for (d0,h0) are emitted right after tile 0-7
            # transposes so they execute while gathers 8-15 are still running.
            ps_p1 = tc.tile_pool(name="ps_p1", bufs=2, space="PSUM")
            psA = ps_p1.__enter__()

            def gather_transpose(ti):
                gx = wp.tile([128, E], BF16, tag="gx")
                nc.gpsimd.indirect_dma_start(
                    out=gx[:],
                    out_offset=None,
                    in_=d_embed[:],
                    in_offset=IndirectOffsetOnAxis(ap=idxf_sb[:, ti : ti + 1], axis=0),
                )
                for k in (0, 1):
                    pt = psA.tile([128, 128], BF16, tag="ptr")
                    nc.tensor.transpose(
                        pt[:], gx[:, k * 128 : (k + 1) * 128], ident128[:]
                    )
                    nc.vector.tensor_copy(xTf[:, k, ti, :], pt[:])

            def proj(d, h):
                for c in range(CH):
                    pj = psA.tile([128, 1024], F32, tag="pj", bufs=3)
                    for k in (0, 1):
                        for nb in (0, 1):
                            nc.tensor.matmul(
                                pj[:, nb * 512 : (nb + 1) * 512],
                                wih_sb[:, d, k, c, :],
                                xTf[:, k, h * 8 + nb * 4 : h * 8 + (nb + 1) * 4, :],
                                start=(k == 0),
                                stop=(k == 1),
                            )
                    # xp^T[t, c, b] = pj + bias (split across ACT and DVE)
                    if (c + h) % 2 == 0:
                        nc.scalar.activation(
                            out=xpT[d][:, h * 128 : (h + 1) * 128, c, :],
                            in_=pj[:].rearrange("p (tt b) -> p tt b", b=BC),
                            func=AF.Identity,
                            bias=xbias_sb[:, d, c : c + 1],
                        )
                    else:
                        nc.vector.tensor_scalar(
                            out=xpT[d][:, h * 128 : (h + 1) * 128, c, :],
                            in0=pj[:].rearrange("p (tt b) -> p tt b", b=BC),
                            scalar1=xbias_sb[:, d, c : c + 1],
                            scalar2=None,
                            op0=ALU.add,
                        )

            for ti in range(8):
                gather_transpose(ti)
            proj(0, 0)
            for ti in range(8, NT):
                gather_transpose(ti)
            proj(1, 1)
            proj(1, 0)
            proj(0, 1)
            ps_p1.__exit__(None, None, None)

            # ---- P2: LSTM recurrence, 4 speculative chains ----
            # (dir, half): half 0 covers steps 0..NH-1 exactly; half 1 warms up
            # from zero state over steps NH-SPEC_W..NH-1 (scratch ping-pong),
            # then runs steps NH..L-1 writing the real hsT slots.
            ps_p2 = tc.tile_pool(name="ps_p2", bufs=6, space="PSUM")
            psB = ps_p2.__enter__()

            def chain_slot(d, hf, j):
                # -> (read_view, write_view, xp_time_index) or None
                if hf == 0:
                    if j >= NH:
                        return None
                    s = j
                    if d == 0:
                        rd, wr = hsT[0][:, :, s, :], hsT[0][:, :, s + 1, :]
                    else:
                        rd, wr = hsT[1][:, :, L - s, :], hsT[1][:, :, L - 1 - s, :]
                else:
                    if j >= NH + SPEC_W:
                        return None
                    if j < SPEC_W:
                        s = NH - SPEC_W + j
                        rd = zro[:] if j == 0 else hwu[d][:, :, (j + 1) % 2, :]
                        wr = hwu[d][:, :, j % 2, :]
                    else:
                        s = NH + (j - SPEC_W)
                        if j == SPEC_W:
                            rd = hwu[d][:, :, (SPEC_W - 1) % 2, :]
                        else:
                            rd = (hsT[0][:, :, s, :] if d == 0
                                  else hsT[1][:, :, L - s, :])
                        wr = (hsT[0][:, :, s + 1, :] if d == 0
                              else hsT[1][:, :, L - 1 - s, :])
                xi = s if d == 0 else L - 1 - s
                return rd, wr, xi

            for j in range(NH + SPEC_W):
                actives = []
                for d, hf in ((0, 0), (1, 0), (0, 1), (1, 1)):
                    cs = chain_slot(d, hf, j)
                    if cs is None:
                        continue
                    rd, wr, xi = cs
                    cd = cst[d][hf]
                    # phase 1: gate matmuls + tanh (ACT queue: th's first)
                    pg = psB.tile([128, CH, BC], F32, tag="pg")
                    nc.tensor.matmul(
                        pg[:].rearrange("p c b -> p (c b)"),
                        ident128[:],
                        xpT[d][:, xi, :, :].rearrange("p c b -> p (c b)"),
                        start=True,
                        stop=False,
                        skip_group_check=True,
                    )
                    for c in range(CH):
                        for k in (0, 1):
                            nc.tensor.matmul(
                                pg[:, c, :],
                                whh_sb[:, d, k, c, :],
                                rd[:, k, :],
                                start=False,
                                stop=(c == CH - 1 and k == 1),
                                skip_group_check=True,
                            )
                    th = lp.tile([128, CH, BC], F32, tag="th", bufs=5)
                    nc.scalar.activation(th[:], pg[:], AF.Tanh)
                    actives.append((d, hf, th, cd, wr))
                # phase 2: v, u, cst on DVE (no h's blocking the queue)
                uvs = []
                for d, hf, th, cd, wr in actives:
                    v = lp.tile([128, 2, BC], F32, tag="v", bufs=5)
                    nc.vector.scalar_tensor_tensor(
                        out=v[:], in0=th[:, 2:4, :], scalar=1.0, in1=cd[:],
                        op0=ALU.add, op1=ALU.mult,
                    )
                    u = lp.tile([128, 2, BC], F32, tag="u", bufs=5)
                    nc.vector.scalar_tensor_tensor(
                        out=u[:], in0=th[:, 0:2, :], scalar=1.0, in1=th[:, 4:6, :],
                        op0=ALU.add, op1=ALU.mult,
                    )
                    nc.vector.scalar_tensor_tensor(
                        out=cd[:], in0=v[:], scalar=0.5, in1=u[:],
                        op0=ALU.mult, op1=ALU.add,
                    )
                # phase 3: tcc tanhs (ACT queue after all th's)
                tccs = []
                for d, hf, th, cd, wr in actives:
                    tcc = lp.tile([128, 2, BC], F32, tag="tcc", bufs=5)
                    nc.scalar.activation(tcc[:], cd[:], AF.Tanh, scale=0.5)
                    tccs.append(tcc)
                # phase 4: h writes
                for (d, hf, th, cd, wr), tcc in zip(actives, tccs):
                    nc.vector.scalar_tensor_tensor(
                        out=wr, in0=th[:, 6:8, :], scalar=1.0,
                        in1=tcc[:], op0=ALU.add, op1=ALU.mult,
                    )
            ps_p2.__exit__(None, None, None)

            # ---- P3: emissions E = exp(sum_d Wout_d @ H_d + bout) ----
            ps_p3 = tc.tile_pool(name="ps_p3", bufs=1, space="PSUM")
            psC = ps_p3.__enter__()
            pf = psC.tile([C, L * BC], F32, tag="pf")
            for d in (0, 1):
                for k in (0, 1):
                    for n in range(4):
                        base = 1 + n * 64 if d == 0 else n * 64
                        nc.tensor.matmul(
                            pf[:, n * 512 : (n + 1) * 512],
                            wout_sb[:, d, k, :],
                            hsT[d][:, k, base : base + 64, :],
                            start=(d == 0 and k == 0),
                            stop=(d == 1 and k == 1),
                        )
            nc.scalar.activation(
                out=eT[:].rearrange("p t b -> p (t b)"),
                in_=pf[:],
                func=AF.Exp,
                bias=bout_sb[:, 0:1],
            )
            # gold emissions: sum_t pf[gold tag] (bout term already via cnt)
            prod2 = pp.tile([C, L, BC], F32, tag="prod2")
            nc.vector.scalar_tensor_tensor(
                out=prod2[:].rearrange("p t b -> p (t b)"), in0=pf[:], scalar=0.0,
                in1=ohcur_tb_sb[:].rearrange("p t b -> p (t b)"), op0=ALU.add, op1=ALU.mult,
            )
            gsum2 = pp.tile([C, BC], F32, tag="gsum2")
            nc.vector.tensor_reduce(
                out=gsum2[:],
                in_=prod2[:].rearrange("p t b -> p b t"),
                axis=AX.X, op=ALU.add,
            )
            nc.gpsimd.tensor_add(gsum[:], gsum[:], gsum2[:])
            # + T[STOP, tag_last]
            stopterm = pp.tile([C, BC], F32, tag="stopterm")
            nc.gpsimd.tensor_tensor(
                out=stopterm[:], in0=ohlast_sb[:],
                in1=tstop_sb[:].to_broadcast([C, BC]), op=ALU.mult,
            )
            nc.gpsimd.tensor_add(gsum[:], gsum[:], stopterm[:])
            ps_p3.__exit__(None, None, None)

            # ---- P5: CRF forward scan, two staggered 4-row chains ----
            ps_p5 = tc.tile_pool(name="ps_p5", bufs=2, space="PSUM")
            psD = ps_p5.__enter__()
            # gold reduce over tags (PE while idle-ish): gold = ones20c^T @ gsum
            pgold = psD.tile([1, BC], F32, tag="pgold", bufs=1)
            nc.tensor.matmul(pgold[:], ones20f[:], gsum[:], start=True, stop=True)
            nc.vector.tensor_copy(gold_sb[:], pgold[:])

            NQ = L // 4
            for j in range(NQ + CSPEC_W):
                for x in range(4):
                    if x == 0:
                        if j >= NQ:
                            continue
                        t = j
                        warm_end = False
                        log_rn = (t % REN == REN - 1)
                    else:
                        if j < CSPEC_W:
                            t = x * NQ - CSPEC_W + j
                            warm_end = (j == CSPEC_W - 1)
                            log_rn = False
                        else:
                            t = x * NQ + (j - CSPEC_W)
                            warm_end = False
                            log_rn = (t % REN == REN - 1)
                    cur = (j + 1) % 2
                    pa = psD.tile([C, BC], F32, tag="pa", bufs=6)
                    nc.tensor.matmul(
                        pa[:], pplus[:], avec[x][:, j % 2, :], start=True, stop=True
                    )
                    nc.vector.scalar_tensor_tensor(
                        out=avec[x][:, cur, :], in0=pa[:], scalar=0.0,
                        in1=eT[:, t, :], op0=ALU.add, op1=ALU.mult,
                    )
                    if log_rn or warm_end:
                        ssum = wp.tile([C, BC], F32, tag=f"ssum{x}")
                        nc.gpsimd.partition_all_reduce(
                            ssum[:], avec[x][:, cur, :], channels=C,
                            reduce_op=bass_isa.ReduceOp.add,
                        )
                        srec = wp.tile([C, BC], F32, tag=f"srec{x}")
                        nc.vector.reciprocal(srec[:], ssum[:])
                        if log_rn:
                            nc.vector.tensor_copy(sall[0:1, :, t // REN], srec[0:1, :])
                        nc.vector.scalar_tensor_tensor(
                            out=avec[x][:, cur, :],
                            in0=avec[x][:, cur, :], scalar=0.0,
                            in1=srec[:], op0=ALU.add, op1=ALU.mult,
                        )

            # ---- P6: ship gold/paf/sall to host (lns done in float64 there;
            # the ACT Ln table is invalid for the ~1e-24 srec magnitudes) ----
            paf = psD.tile([1, BC], F32, tag="paf", bufs=1)
            nc.tensor.matmul(
                paf[:], wstop[:], avec[3][:, (NQ + CSPEC_W) % 2, :],
                start=True, stop=True,
            )
            paf_sb = wp.tile([1, BC], F32, tag="paf_sb")
            nc.vector.tensor_copy(paf_sb[:], paf[:])
            nc.sync.dma_start(out=d_gold[:], in_=gold_sb[:])
            nc.sync.dma_start(out=d_paf[:], in_=paf_sb[:])
            nc.sync.dma_start(out=d_sall[:], in_=sall[:])
            if DEBUG:
                nc.sync.dma_start(out=d_dbg_eT[:], in_=eT[:])
                nc.sync.dma_start(out=d_dbg_gsum[:], in_=gsum[:])
                nc.sync.dma_start(out=d_dbg_gold[:], in_=gold_sb[:])
                nc.sync.dma_start(out=d_dbg_hf[:], in_=hsT[0][:, :, L, :])
                nc.sync.dma_start(out=d_dbg_hb[:], in_=hsT[1][:, :, 0, :])
                nc.sync.dma_start(out=d_dbg_sall[:], in_=sall[:])
                nc.sync.dma_start(out=d_dbg_xp[:], in_=xpT[0][:, 0:8, :, :])
                for d in (0, 1):
                    nc.sync.dma_start(out=d_dbg_cst[:, d, :, :], in_=cst[d][:])
            ps_p5.__exit__(None, None, None)

    nc.finalize()
    return nc


def _prep_inmaps(inputs):
    bf = ml_dtypes.bfloat16
    sent = np.asarray(inputs["sentences"])
    tags = np.asarray(inputs["tags"])
    embed = np.asarray(inputs["embed"], dtype=np.float32)
    trans = np.asarray(inputs["transitions"], dtype=np.float32)
    h0 = np.asarray(inputs["h0"], dtype=np.float32)
    c0 = np.asarray(inputs["c0"], dtype=np.float32)
    W_out = np.asarray(inputs["W_out"], dtype=np.float32)
    b_out = np.asarray(inputs["b_out"], dtype=np.float32)

    rs = np.full((G, 1), 0.5, np.float32)
    rs[2 * H : 3 * H] = 1.0  # g-gate rows unscaled

    embed_bf = np.ascontiguousarray(embed.astype(bf))

    def chunk_weights(W):  # W [G, K_in] -> [128, 2, CH, 128] = [p, k, c, m]
        Kin = W.shape[1]
        Wr = W.reshape(4, 2, 128, Kin // 128, 128)  # [gate, hh, m, k, p]
        return np.ascontiguousarray(Wr.transpose(4, 3, 0, 1, 2).reshape(128, Kin // 128, CH, 128))

    wih = np.zeros((128, 2, 2, CH, 128), np.float32)
    whh = np.zeros((128, 2, 2, CH, 128), np.float32)
    xbias = np.zeros((128, 2, CH), np.float32)
    for d, (Wih, Whh, b) in enumerate(
        [
            (inputs["Wih_f"], inputs["Whh_f"], inputs["b_f"]),
            (inputs["Wih_b"], inputs["Whh_b"], inputs["b_b"]),
        ]
    ):
        Wih = np.asarray(Wih, np.float32) * rs
        Whh = np.asarray(Whh, np.float32) * rs * 0.5
        bt = np.asarray(b, np.float32) * rs[:, 0]
        wih[:, d] = chunk_weights(Wih)
        whh[:, d] = chunk_weights(Whh)
        xbias[:, d] = bt.reshape(4, 2, 128).transpose(2, 0, 1).reshape(128, CH)
    wih = np.ascontiguousarray(wih.astype(bf))
    whh = np.ascontiguousarray(whh.astype(bf))

    # wout^T [p, d, k, m] = 0.5 * W_out[m, d*256 + k*128 + p]
    wout = np.ascontiguousarray(
        (0.5 * W_out).reshape(C, 2, 2, 128).transpose(3, 1, 2, 0).astype(bf)
    )
    bout = np.ascontiguousarray(b_out[:, None])
    transT = np.ascontiguousarray(trans.T)
    transTb = np.ascontiguousarray(trans.T.astype(bf))
    tstop = np.ascontiguousarray(trans[STOP, :][:, None])

    in_maps = []
    for q in range(NCORES):
        bs = slice(q * BC, (q + 1) * BC)
        sq = sent[bs]  # [BC, L]
        tq = tags[bs]
        idx_f = np.ascontiguousarray(
            sq.T.reshape(NT, TPT, BC).transpose(1, 2, 0).reshape(128, NT).astype(np.int32)
        )
        h0q = np.ascontiguousarray(
            (2.0 * h0[:, bs, :]).reshape(2, BC, 2, 128).transpose(3, 0, 2, 1).astype(bf)
        )
        c0q = np.ascontiguousarray(
            (2.0 * c0[:, bs, :]).reshape(2, BC, 2, 128).transpose(3, 0, 2, 1).astype(np.float32)
        )
        te_prev = np.concatenate(
            [np.full((BC, 1), START, tags.dtype), tq[:, :-1]], axis=1
        )  # prev tag at each t
        ar = np.arange(C)
        ohprev = (ar[:, None, None] == te_prev[None, :, :]).astype(np.float32)
        ohcur = (ar[:, None, None] == tq[None, :, :]).astype(np.float32)
        ohcur_tb = np.ascontiguousarray(ohcur.transpose(0, 2, 1))
        ohlast = (ar[:, None] == tq[None, :, L - 1]).astype(np.float32)
        a0 = ((ar[:, None] == START) * np.ones((1, BC))).astype(bf)
        in_maps.append(
            {
                "embed_bf": embed_bf,
                "idx_f": idx_f,
                "wih": wih,
                "whh": whh,
                "xbias": xbias,
                "h0T": h0q,
                "c0T": c0q,
                "woutT": wout,
                "bout": bout,
                "transT": transT,
                "transTb": transTb,
                "tstop": tstop,
                "ohprev": np.ascontiguousarray(ohprev.astype(bf)),
                "ohcur": np.ascontiguousarray(ohcur),
                "ohcur_tb": ohcur_tb,
                "ohlast": np.ascontiguousarray(ohlast),
                "a0": np.ascontiguousarray(a0),
            }
        )
    return in_maps


def get_module():
    if "nc" not in _CACHE:
        _CACHE["nc"] = _build_module()
    return _CACHE["nc"]


def _finalize(outs):
    """Host-side: partial = sum_b [ln(paf_b) - sum_r ln(srec_br) - gold_b]."""
    paf = np.asarray(outs["paf_out"], np.float64)[0]
    sall = np.asarray(outs["sall_out"], np.float64)[0]
    gold = np.asarray(outs["gold_out"], np.float64)[0]
    F = np.log(paf) - np.log(sall).sum(axis=1)
    return float((F - gold).sum())


def kernel(**inputs):
    nc = get_module()
    in_maps = _prep_inmaps(inputs)
    res = run_bass_kernel_spmd(nc, in_maps, core_ids=list(range(NCORES)))
    total = sum(_finalize(r) for r in res.results)
    return np.float32(total / B)
